# revision 15
# baseline (speedup 1.0000x reference)
"""Trainium2 Bass kernel for a 2-layer GATv2 + top-k pooling + classifier.

Distribution (8 NeuronCores): partition nodes (and their incoming edges)
across cores; layer-1 source features are computed replicated (x is
replicated), layer-2 source features are exchanged with one AllGather.
GAT weights / classifier are replicated.

Self-contained: only needs concourse (Bass), numpy, ml_dtypes.
"""

import numpy as np
import ml_dtypes

import concourse.bass as bass
import concourse.bacc as bacc
import concourse.mybir as mybir
import concourse.tile as tile
from concourse.bass import AP
from concourse.bass_utils import run_bass_kernel_spmd
from concourse.masks import make_identity

P = 128
NCORES = 8
SPLIT = 32768          # int16 gather index limit per table
NEG_SLOPE = 0.2
TOPK = 10

bf16 = mybir.dt.bfloat16
f32 = mybir.dt.float32
i16 = mybir.dt.int16

_bf = ml_dtypes.bfloat16


def _wrap_idx(idx):
    """Pack int16 indices into dma_gather's [128, n//16] SBUF layout."""
    n = idx.shape[0]
    assert n % 16 == 0
    t = idx.astype(np.int16).reshape(n // 16, 16).T
    return np.tile(t, (8, 1))


def _pad(a, n, val=0):
    out = np.full(n, val, dtype=np.int64)
    out[: len(a)] = a
    return out


def _ceil(a, b):
    return -(-a // b)


def _bcast_mid(ap, ct):
    """[P, d] AP -> [P, ct, d] AP with broadcast (step-0) middle dim."""
    return AP(ap.tensor, ap.offset, [ap.ap[0], [0, ct], ap.ap[1]])


def _prep_graph(src, dst, n_nodes):
    """Partition edges by dst core, sort by dst, tile dsts by 128, split
    sources at SPLIT. Returns per-core packed index/selection inputs plus
    the (shared) per-tile chunk counts."""
    npc = n_nodes // NCORES                # nodes per core
    ntile = _ceil(npc, P)                  # dst tiles per core
    core_of = dst // npc

    pc = []
    for c in range(NCORES):
        m = core_of == c
        es = src[m].astype(np.int64)
        ed = dst[m].astype(np.int64) - c * npc
        o = np.argsort(ed, kind="stable")
        es, ed = es[o], ed[o]
        bounds = np.searchsorted(ed, np.arange(0, ntile * P + 1, P))
        tiles = []
        for t in range(ntile):
            sl = slice(bounds[t], bounds[t + 1])
            ts_, td_ = es[sl], ed[sl]
            am = ts_ < SPLIT
            tiles.append(dict(
                a_src=ts_[am], a_fd=td_[am], a_col=td_[am] - t * P,
                b_src=ts_[~am] - SPLIT, b_fd=td_[~am], b_col=td_[~am] - t * P,
            ))
        pc.append(tiles)

    cA = [max(_ceil(len(pc[c][t]["a_src"]), P) for c in range(NCORES))
          for t in range(ntile)]
    cB = [max(_ceil(len(pc[c][t]["b_src"]), P) for c in range(NCORES))
          for t in range(ntile)]
    C = [cA[t] + cB[t] for t in range(ntile)]

    per_core = []
    for c in range(NCORES):
        iA, iB, iF, Ss = [], [], [], []
        for t in range(ntile):
            d = pc[c][t]
            nA, nB = len(d["a_src"]), len(d["b_src"])
            if cA[t]:
                iA.append(_wrap_idx(_pad(d["a_src"], cA[t] * P)))
            if cB[t]:
                iB.append(_wrap_idx(_pad(d["b_src"], cB[t] * P)))
            S3 = np.zeros((C[t] * P, P), dtype=np.float32)
            S3[np.arange(nA), d["a_col"]] = 1.0
            S3[cA[t] * P + np.arange(nB), d["b_col"]] = 1.0
            Ss.append(
                S3.reshape(C[t], P, P).transpose(1, 0, 2).reshape(P, C[t] * P))
            iF.append(
                S3.reshape(C[t], P, P).transpose(2, 0, 1).reshape(P, C[t] * P))
        per_core.append(dict(
            iA=np.concatenate(iA, axis=1) if iA else np.zeros((P, 8), np.int16),
            iB=np.concatenate(iB, axis=1) if iB else np.zeros((P, 8), np.int16),
            ST=np.concatenate(iF, axis=1).astype(_bf),
            S=np.concatenate(Ss, axis=1).astype(_bf),
        ))
    return per_core, cA, cB, C, ntile, npc


def _aug(w, b):
    return np.vstack([np.asarray(w), np.asarray(b)[None, :]])


def build_nc(meta, stop_after="full"):
    n_nodes = meta["n_nodes"]
    npc = meta["npc"]
    ntile = meta["ntile"]
    cA, cB, C = meta["cA"], meta["cB"], meta["C"]
    Cmax = max(C)
    sumA, sumB, sumC = sum(cA), sum(cB), sum(C)
    emb = meta["emb"]
    dd = meta["d"]
    H = meta["H"]
    F = dd // H
    ncls = meta["ncls"]
    npc_pad = ntile * P
    nfull = meta["nfull_pad"]
    rowsA = min(nfull, SPLIT)
    rowsB_pad = max(_ceil(nfull - rowsA, P) * P, P)
    ngrp = npc // TOPK
    use_B = n_nodes > SPLIT
    nfull_ag_pad = _ceil(n_nodes, P) * P

    nc = bacc.Bacc(num_swdge_queues=4)

    xTl = nc.declare_dram_parameter("xTl", [emb + 1, npc_pad], bf16, isOutput=False)
    w1s = nc.declare_dram_parameter("w1s", [emb + 1, dd], bf16, isOutput=False)
    w1d = nc.declare_dram_parameter("w1d", [emb + 1, dd], bf16, isOutput=False)
    w2s = nc.declare_dram_parameter("w2s", [dd + 1, dd], bf16, isOutput=False)
    w2d = nc.declare_dram_parameter("w2d", [dd + 1, dd], bf16, isOutput=False)
    a1r = nc.declare_dram_parameter("a1r", [P, dd], bf16, isOutput=False)
    a2r = nc.declare_dram_parameter("a2r", [P, dd], bf16, isOutput=False)
    wca = nc.declare_dram_parameter("wca", [dd + 1, ncls], f32, isOutput=False)
    pwp = nc.declare_dram_parameter("pwp", [P, 5], f32, isOutput=False)
    iA_in = nc.declare_dram_parameter("iA", [P, max(sumA, 1) * 8], i16, isOutput=False)
    iB_in = nc.declare_dram_parameter("iB", [P, max(sumB, 1) * 8], i16, isOutput=False)
    ST_in = nc.declare_dram_parameter("ST", [P, sumC * P], bf16, isOutput=False)
    S_in = nc.declare_dram_parameter("S", [P, sumC * P], bf16, isOutput=False)
    out = nc.declare_dram_parameter("out", [ngrp, ncls], f32, isOutput=True)

    fd1 = nc.dram_tensor("fd1", [npc_pad, dd], bf16)
    fd2 = nc.dram_tensor("fd2", [npc_pad, dd], bf16)
    fs1l = nc.dram_tensor("fs1l", [npc, dd], bf16)
    fs1f = nc.dram_tensor("fs1f", [nfull_ag_pad, dd], bf16, addr_space="Shared")
    fs2l = nc.dram_tensor("fs2l", [npc, dd], bf16)
    fs2f = nc.dram_tensor("fs2f", [nfull_ag_pad, dd], bf16, addr_space="Shared")

    AF = mybir.ActivationFunctionType
    ALU = mybir.AluOpType
    BLK = 1024

    with tile.TileContext(nc) as tc:
        with (
            tc.tile_pool(name="const", bufs=1) as cpool,
            tc.tile_pool(name="wpool", bufs=1) as wpool,
            tc.tile_pool(name="xload", bufs=2) as xpool,
            tc.tile_pool(name="mmout", bufs=3) as mpool,
            tc.tile_pool(name="edgeg", bufs=3) as epool,
            tc.tile_pool(name="vp", bufs=2) as vpool,
            tc.tile_pool(name="sp2", bufs=2) as s2pool,
            tc.tile_pool(name="zp", bufs=2) as zpool,
            tc.tile_pool(name="small", bufs=3) as spool,
            tc.tile_pool(name="hbuf", bufs=1) as hpool,
            tc.tile_pool(name="psA", bufs=2, space="PSUM") as psA,
            tc.tile_pool(name="psT", bufs=2, space="PSUM") as psT,
            tc.tile_pool(name="psE", bufs=2, space="PSUM") as psE,
            tc.tile_pool(name="psF", bufs=2, space="PSUM") as psF,
        ):
            ones1 = cpool.tile([1, P], bf16)
            nc.vector.memset(ones1[:], 1.0)
            ones1f = cpool.tile([1, P], f32)
            nc.vector.memset(ones1f[:], 1.0)
            ident = cpool.tile([P, P], bf16)
            make_identity(nc, ident[:])
            a1t = cpool.tile([P, dd], bf16)
            nc.sync.dma_start(out=a1t[:], in_=a1r[:])
            a2t = cpool.tile([P, dd], bf16)
            nc.sync.dma_start(out=a2t[:], in_=a2r[:])
            pw = cpool.tile([P, 5], f32)
            nc.sync.dma_start(out=pw[:], in_=pwp[:])

            def load_w(src_t, kdim, nm):
                t0 = wpool.tile([P, dd], bf16, tag=nm + "0")
                t1 = wpool.tile([P, dd], bf16, tag=nm + "1")
                t2 = wpool.tile([1, dd], bf16, tag=nm + "2")
                nc.sync.dma_start(out=t0[:], in_=src_t[0:P, :])
                nc.sync.dma_start(out=t1[:], in_=src_t[P:2 * P, :])
                nc.sync.dma_start(out=t2[:], in_=src_t[kdim:kdim + 1, :])
                return t0, t1, t2

            w1s_t = load_w(w1s, emb, "w1s")
            w1d_t = load_w(w1d, emb, "w1d")
            w2s_t = load_w(w2s, dd, "w2s")
            w2d_t = load_w(w2d, dd, "w2d")

            def mm_rows(x0, x1, m0, wtile, psum):
                nc.tensor.matmul(out=psum[:], lhsT=x0[:, m0:m0 + P],
                                 rhs=wtile[0][:], start=True, stop=False)
                nc.tensor.matmul(out=psum[:], lhsT=x1[:, m0:m0 + P],
                                 rhs=wtile[1][:], start=False, stop=False)
                nc.tensor.matmul(out=psum[:], lhsT=ones1[:], rhs=wtile[2][:],
                                 start=False, stop=True)

            # ---------- phase A: fs1 local -> AllGather, fd1 local overlaps --
            def lin_phase(wtile, dst_t, dst_rows):
                for b in range(_ceil(npc_pad, BLK)):
                    w = min(BLK, npc_pad - b * BLK)
                    x0 = xpool.tile([P, BLK], bf16, tag="x0")
                    x1 = xpool.tile([P, BLK], bf16, tag="x1")
                    nc.sync.dma_start(out=x0[:, :w],
                                      in_=xTl[0:P, b * BLK:b * BLK + w])
                    nc.sync.dma_start(out=x1[:, :w],
                                      in_=xTl[P:2 * P, b * BLK:b * BLK + w])
                    for m in range(w // P):
                        row0 = b * BLK + m * P
                        ps = psA.tile([P, dd], f32, space="PSUM", tag="psa")
                        mm_rows(x0, x1, m * P, wtile, ps)
                        ot = mpool.tile([P, dd], bf16, tag="fsout")
                        nc.scalar.copy(out=ot[:], in_=ps[:])
                        hi = min(row0 + P, dst_rows)
                        if hi > row0:
                            nc.sync.dma_start(out=dst_t[row0:hi, :],
                                              in_=ot[: hi - row0, :])

            lin_phase(w1s_t, fs1l, npc)
            lin_phase(w1d_t, fd1, npc_pad)
            nc.gpsimd.collective_compute(
                "AllGather", ALU.bypass,
                replica_groups=[list(range(NCORES))],
                ins=[fs1l[:]], outs=[fs1f[0:n_nodes, :]])

            # ---------- edge phase ----------
            h1 = hpool.tile([P, ntile, dd], bf16, tag="h")
            h2 = hpool.tile([P, ntile, dd], bf16, tag="h")

            def edge_phase(tabA, tabB, tabF, a_t, hdst, scale_posw, depth="all"):
                offA = offB = offC = 0
                for t in range(ntile):
                    ca, cb, ct = cA[t], cB[t], C[t]
                    St = s2pool.tile([P, Cmax * P], bf16, tag="S")
                    nc.sync.dma_start(
                        out=St[:, : ct * P],
                        in_=S_in[:, offC * P:(offC + ct) * P])
                    E = epool.tile([P, Cmax, dd], bf16, tag="E")
                    if ca:
                        ia = spool.tile([P, Cmax * 8], i16, tag="ia")
                        nc.sync.dma_start(
                            out=ia[:, : ca * 8],
                            in_=iA_in[:, offA * 8:(offA + ca) * 8])
                        ah = (ca + 1) // 2
                        nc.gpsimd.dma_gather(
                            out_ap=E[:, 0:ah, :], in_ap=tabA,
                            idxs_ap=ia[:, : ah * 8], num_idxs=ah * P,
                            num_idxs_reg=ah * P, elem_size=dd,
                            single_packet=False, queue_num=(3 * t) % 4)
                        if ca > ah:
                            nc.gpsimd.dma_gather(
                                out_ap=E[:, ah:ca, :], in_ap=tabA,
                                idxs_ap=ia[:, ah * 8: ca * 8],
                                num_idxs=(ca - ah) * P,
                                num_idxs_reg=(ca - ah) * P, elem_size=dd,
                                single_packet=False, queue_num=(3 * t + 1) % 4)
                    if cb:
                        ib = spool.tile([P, Cmax * 8], i16, tag="ib")
                        nc.sync.dma_start(
                            out=ib[:, : cb * 8],
                            in_=iB_in[:, offB * 8:(offB + cb) * 8])
                        nc.gpsimd.dma_gather(
                            out_ap=E[:, ca:ct, :], in_ap=tabB,
                            idxs_ap=ib[:, : cb * 8], num_idxs=cb * P,
                            num_idxs_reg=cb * P, elem_size=dd,
                            single_packet=False, queue_num=(3 * t + 2) % 4)
                    # fd broadcast: fd rows for this tile's 128 dsts, expanded
                    # to edge slots via the transposed one-hot (PE matmul)
                    STt = s2pool.tile([P, Cmax * P], bf16, tag="ST")
                    nc.sync.dma_start(
                        out=STt[:, : ct * P],
                        in_=ST_in[:, offC * P:(offC + ct) * P])
                    fdt = spool.tile([P, dd], bf16, tag="fdt")
                    nc.sync.dma_start(out=fdt[:], in_=tabF[t * P:(t + 1) * P, :])
                    w_ = ct * dd
                    LZ = zpool.tile([P, Cmax * dd], bf16, tag="LZ")
                    G = 2
                    for g0 in range(0, ct, G):
                        gn = min(G, ct - g0)
                        psf = psF.tile([P, G, dd], f32, space="PSUM", tag="psf")
                        for i in range(gn):
                            nc.tensor.matmul(
                                out=psf[:, i, :],
                                lhsT=STt[:, (g0 + i) * P:(g0 + i + 1) * P],
                                rhs=fdt[:], start=True, stop=False)
                            nc.tensor.matmul(
                                out=psf[:, i, :], lhsT=ident[:],
                                rhs=E[:, g0 + i, :],
                                start=False, stop=True)
                        nc.scalar.activation(
                            LZ[:, g0 * dd:(g0 + gn) * dd].rearrange(
                                "p (g d) -> p g d", d=dd),
                            psf[:, 0:gn, :], AF.Prelu, alpha=NEG_SLOPE)

                    if depth == "g":
                        nc.vector.memset(hdst[:, t, :], 0.0)
                        offA += ca; offB += cb; offC += ct
                        continue
                    T = zpool.tile([P, Cmax * dd], bf16, tag="T")
                    nc.vector.tensor_mul(
                        out=T[:, :w_].rearrange("p (c d) -> p c d", d=dd),
                        in0=LZ[:, :w_].rearrange("p (c d) -> p c d", d=dd),
                        in1=_bcast_mid(a_t[:], ct))
                    TF = zpool.tile([P, Cmax * dd // 2], bf16, tag="TF")
                    tv = T[:, :w_].rearrange("p (ch f) -> p ch f", f=F)
                    nc.vector.tensor_add(
                        out=TF[:, : w_ // 2].rearrange("p (ch f) -> p ch f",
                                                       f=F // 2),
                        in0=tv[:, :, 0:F // 2], in1=tv[:, :, F // 2:F])
                    TF2 = zpool.tile([P, Cmax * dd // 4], bf16, tag="TF2")
                    t2v = TF[:, : w_ // 2].rearrange("p (ch f) -> p ch f",
                                                     f=F // 2)
                    nc.vector.tensor_add(
                        out=TF2[:, : w_ // 4].rearrange("p (ch f) -> p ch f",
                                                        f=F // 4),
                        in0=t2v[:, :, 0:F // 4], in1=t2v[:, :, F // 4:F // 2])
                    score = spool.tile([P, Cmax * H], f32, tag="sc")
                    nc.vector.reduce_sum(
                        out=score[:, : ct * H],
                        in_=TF2[:, : w_ // 4].rearrange("p (ch f) -> p ch f",
                                                        f=F // 4),
                        axis=mybir.AxisListType.X)
                    EX = spool.tile([P, Cmax * H], f32, tag="ex")
                    nc.scalar.activation(EX[:, : ct * H], score[:, : ct * H],
                                         AF.Exp)
                    if depth == "dve":
                        nc.vector.memset(hdst[:, t, :], 0.0)
                        offA += ca; offB += cb; offC += ct
                        continue

                    V = vpool.tile([P, Cmax, dd + H], bf16, tag="V")
                    EXB = zpool.tile([P, Cmax * dd], bf16, tag="EXB")
                    exs = EX[:, : ct * H]
                    nc.scalar.copy(
                        out=EXB[:, :w_].rearrange("p (ch f) -> p ch f", f=F),
                        in_=AP(exs.tensor, exs.offset,
                               [exs.ap[0], exs.ap[1], [0, F]]))
                    nc.vector.tensor_mul(
                        out=V[:, 0:ct, 0:dd],
                        in0=E[:, 0:ct, :],
                        in1=EXB[:, :w_].rearrange("p (c d) -> p c d", d=dd))
                    nc.scalar.copy(
                        out=V[:, 0:ct, dd:dd + H],
                        in_=EX[:, : ct * H].rearrange("p (c h) -> p c h", h=H))

                    if depth == "v":
                        nc.vector.memset(hdst[:, t, :], 0.0)
                        offA += ca; offB += cb; offC += ct
                        continue
                    agg = psE.tile([P, dd + H], f32, space="PSUM", tag="agg")
                    for c in range(ct):
                        nc.tensor.matmul(
                            out=agg[:], lhsT=St[:, c * P:(c + 1) * P],
                            rhs=V[:, c, :], start=(c == 0), stop=(c == ct - 1))

                    den = spool.tile([P, H], f32, tag="den")
                    nc.vector.tensor_scalar_max(den[:], agg[:, dd:dd + H], 1e-9)
                    rec = spool.tile([P, H], f32, tag="rec")
                    nc.vector.reciprocal(rec[:], den[:])
                    if scale_posw:
                        nc.vector.tensor_scalar_mul(rec[:], rec[:],
                                                    pw[:, t % 5:t % 5 + 1])
                    for h in range(H):
                        nc.vector.tensor_scalar_mul(
                            hdst[:, t, h * F:(h + 1) * F],
                            agg[:, h * F:(h + 1) * F], rec[:, h:h + 1])
                    offA += ca
                    offB += cb
                    offC += ct

            if stop_after != "A":
                depth = {"L1g": "g", "L1dve": "dve", "L1v": "v"}.get(
                    stop_after, "all")
                if use_B:
                    edge_phase(fs1f[0:SPLIT, :], fs1f[SPLIT:nfull_ag_pad, :],
                               fd1, a1t, h1, False, depth)
                else:
                    edge_phase(fs1f[0:nfull_ag_pad, :], fs1f[:],
                               fd1, a1t, h1, False, depth)

            # ---------- transpose h1 ----------
            hT0 = hpool.tile([P, npc_pad], bf16, tag="t0")
            hT1 = hpool.tile([P, npc_pad], bf16, tag="t1")
            for t in range(ntile if stop_after in ("TR", "FS2", "AG", "full") else 0):
                for half, ht in ((0, hT0), (1, hT1)):
                    pt = psT.tile([P, P], bf16, space="PSUM", tag="ptr")
                    nc.tensor.transpose(
                        out=pt[:], in_=h1[:, t, half * P:(half + 1) * P],
                        identity=ident[:])
                    nc.scalar.copy(out=ht[:, t * P:(t + 1) * P], in_=pt[:])

            # ---------- fs2 local -> AllGather, fd2 overlaps ----------
            do_l2 = stop_after in ("FS2", "AG", "full")

            def lin2_phase(wt, dst_t, dst_rows):
                for t in range(ntile):
                    ps = psA.tile([P, dd], f32, space="PSUM", tag="psa")
                    nc.tensor.matmul(out=ps[:], lhsT=hT0[:, t * P:(t + 1) * P],
                                     rhs=wt[0][:], start=True, stop=False)
                    nc.tensor.matmul(out=ps[:], lhsT=hT1[:, t * P:(t + 1) * P],
                                     rhs=wt[1][:], start=False, stop=False)
                    nc.tensor.matmul(out=ps[:], lhsT=ones1[:], rhs=wt[2][:],
                                     start=False, stop=True)
                    ot = mpool.tile([P, dd], bf16, tag="fsout")
                    nc.scalar.copy(out=ot[:], in_=ps[:])
                    hi = min((t + 1) * P, dst_rows)
                    if hi > t * P:
                        nc.sync.dma_start(out=dst_t[t * P:hi, :],
                                          in_=ot[: hi - t * P, :])

            if do_l2:
                lin2_phase(w2s_t, fs2l, npc)
                lin2_phase(w2d_t, fd2, npc_pad)

            # ---------- AllGather fs2 ----------
            do_rest = stop_after in ("AG", "full")
            if do_rest:
                nc.gpsimd.collective_compute(
                "AllGather", ALU.bypass,
                    replica_groups=[list(range(NCORES))],
                    ins=[fs2l[:]], outs=[fs2f[0:n_nodes, :]])

            if do_rest and stop_after == "AG":
                do_rest = False
            if do_rest and use_B:
                edge_phase(fs2f[0:SPLIT, :], fs2f[SPLIT:nfull_ag_pad, :],
                           fd2, a2t, h2, True)
            elif do_rest:
                edge_phase(fs2f[0:nfull_ag_pad, :], fs2f[:], fd2, a2t, h2, True)

            # ---------- transpose h2 ----------
            gT0 = hpool.tile([P, npc_pad], bf16, tag="t0")
            gT1 = hpool.tile([P, npc_pad], bf16, tag="t1")
            for t in range(ntile if stop_after == "full" else 0):
                for half, ht in ((0, gT0), (1, gT1)):
                    pt = psT.tile([P, P], bf16, space="PSUM", tag="ptr")
                    nc.tensor.transpose(
                        out=pt[:], in_=h2[:, t, half * P:(half + 1) * P],
                        identity=ident[:])
                    nc.scalar.copy(out=ht[:, t * P:(t + 1) * P], in_=pt[:])

            # ---------- pooling + classifier ----------
            p0 = hpool.tile([P, npc // TOPK], f32, tag="p0")
            p1 = hpool.tile([P, npc // TOPK], f32, tag="p1")
            if stop_after == "full":
                nc.vector.reduce_sum(
                    out=p0[:],
                    in_=gT0[:, :npc].rearrange("p (g k) -> p g k", k=TOPK),
                    axis=mybir.AxisListType.X)
                nc.vector.reduce_sum(
                    out=p1[:],
                    in_=gT1[:, :npc].rearrange("p (g k) -> p g k", k=TOPK),
                    axis=mybir.AxisListType.X)
            else:
                nc.vector.memset(p0[:], 0.0)
                nc.vector.memset(p1[:], 0.0)

            wc0 = wpool.tile([P, ncls], f32, tag="wc0")
            wc1 = wpool.tile([P, ncls], f32, tag="wc1")
            wc2 = wpool.tile([1, ncls], f32, tag="wc2")
            nc.sync.dma_start(out=wc0[:], in_=wca[0:P, :])
            nc.sync.dma_start(out=wc1[:], in_=wca[P:2 * P, :])
            nc.sync.dma_start(out=wc2[:], in_=wca[dd:dd + 1, :])

            for g0 in range(0, ngrp, P):
                gw = min(P, ngrp - g0)
                pc_ = psA.tile([P, ncls], f32, space="PSUM", tag="psa")
                nc.tensor.matmul(out=pc_[:gw, :], lhsT=p0[:, g0:g0 + gw],
                                 rhs=wc0[:], start=True, stop=False)
                nc.tensor.matmul(out=pc_[:gw, :], lhsT=p1[:, g0:g0 + gw],
                                 rhs=wc1[:], start=False, stop=False)
                nc.tensor.matmul(out=pc_[:gw, :], lhsT=ones1f[:, :gw],
                                 rhs=wc2[:], start=False, stop=True)
                oc = mpool.tile([P, ncls], f32, tag="ocls")
                nc.scalar.copy(out=oc[:gw, :], in_=pc_[:gw, :])
                nc.sync.dma_start(out=out[g0:g0 + gw, :], in_=oc[:gw, :])

    nc.compile()
    return nc


def _build_inputs(inputs):
    x = np.asarray(inputs["x"], dtype=np.float32)
    src = np.asarray(inputs["src"]).astype(np.int64)
    dst = np.asarray(inputs["dst"]).astype(np.int64)
    n_nodes, emb = x.shape
    dd = np.asarray(inputs["w1_src"]).shape[1]
    H = np.asarray(inputs["a1"]).shape[0]
    ncls = np.asarray(inputs["wc"]).shape[1]

    per_core, cA, cB, C, ntile, npc = _prep_graph(src, dst, n_nodes)
    npc_pad = ntile * P
    nfull_pad = _ceil(n_nodes, P) * P

    meta = dict(n_nodes=n_nodes, npc=npc, ntile=ntile, cA=cA, cB=cB, C=C,
                emb=emb, d=dd, H=H, ncls=ncls, nfull_pad=nfull_pad)

    w1s = _aug(inputs["w1_src"], inputs["b1_src"]).astype(_bf)
    w1d = _aug(inputs["w1_dst"], inputs["b1_dst"]).astype(_bf)
    w2s = _aug(inputs["w2_src"], inputs["b2_src"]).astype(_bf)
    w2d = _aug(inputs["w2_dst"], inputs["b2_dst"]).astype(_bf)
    a1rr = np.tile(np.asarray(inputs["a1"]).reshape(1, -1), (P, 1)).astype(_bf)
    a2rr = np.tile(np.asarray(inputs["a2"]).reshape(1, -1), (P, 1)).astype(_bf)
    wca = _aug(inputs["wc"], inputs["bc"]).astype(np.float32)
    pos_w = np.asarray(inputs["pos_w"], dtype=np.float32)
    pwp = np.zeros((P, 5), dtype=np.float32)
    for j in range(5):
        for p in range(P):
            pwp[p, j] = pos_w[(P * j + p) % TOPK]

    in_maps = []
    for c in range(NCORES):
        d = per_core[c]
        xl = np.zeros((emb + 1, npc_pad), dtype=_bf)
        xl[:emb, :npc] = x[c * npc:(c + 1) * npc].T.astype(_bf)
        xl[emb, :] = _bf(1.0)
        in_maps.append(dict(
            xTl=xl, w1s=w1s, w1d=w1d, w2s=w2s, w2d=w2d,
            a1r=a1rr, a2r=a2rr, wca=wca, pwp=pwp,
            iA=np.ascontiguousarray(d["iA"]), iB=np.ascontiguousarray(d["iB"]),
            ST=np.ascontiguousarray(d["ST"]), S=np.ascontiguousarray(d["S"]),
        ))
    return meta, in_maps


def run(inputs, trace=False, stop_after="full", cores=None):
    meta, in_maps = _build_inputs(inputs)
    nc = build_nc(meta, stop_after=stop_after)
    ids = list(range(NCORES)) if cores is None else list(range(cores))
    res = run_bass_kernel_spmd(nc, [in_maps[c] for c in ids], core_ids=ids,
                               trace=trace)
    outs = [res.results[i]["out"] for i in range(len(ids))]
    return np.concatenate(outs, axis=0), res


def kernel(**inputs):
    out, _ = run(inputs, trace=False)
    return out



# revision 17
# speedup vs baseline: 1.1283x; 1.1283x over previous
"""Trainium2 Bass kernel for a 2-layer GATv2 + top-k pooling + classifier.

Distribution (8 NeuronCores): partition nodes (and their incoming edges)
across cores; layer-1 source features are computed replicated (x is
replicated), layer-2 source features are exchanged with one AllGather.
GAT weights / classifier are replicated.

Self-contained: only needs concourse (Bass), numpy, ml_dtypes.
"""

import numpy as np
import ml_dtypes

import concourse.bass as bass
import concourse.bacc as bacc
import concourse.mybir as mybir
import concourse.tile as tile
from concourse.bass import AP
from concourse.bass_utils import run_bass_kernel_spmd
from concourse.masks import make_identity

P = 128
NCORES = 8
SPLIT = 32768          # int16 gather index limit per table
NEG_SLOPE = 0.2
TOPK = 10

bf16 = mybir.dt.bfloat16
f32 = mybir.dt.float32
i16 = mybir.dt.int16

_bf = ml_dtypes.bfloat16


def _wrap_idx(idx):
    """Pack int16 indices into dma_gather's [128, n//16] SBUF layout."""
    n = idx.shape[0]
    assert n % 16 == 0
    t = idx.astype(np.int16).reshape(n // 16, 16).T
    return np.tile(t, (8, 1))


def _pad(a, n, val=0):
    out = np.full(n, val, dtype=np.int64)
    out[: len(a)] = a
    return out


def _ceil(a, b):
    return -(-a // b)


def _bcast_mid(ap, ct):
    """[P, d] AP -> [P, ct, d] AP with broadcast (step-0) middle dim."""
    return AP(ap.tensor, ap.offset, [ap.ap[0], [0, ct], ap.ap[1]])


def _prep_graph(src, dst, n_nodes):
    """Partition edges by dst core, sort by dst, tile dsts by 128, split
    sources at SPLIT. Returns per-core packed index/selection inputs plus
    the (shared) per-tile chunk counts."""
    npc = n_nodes // NCORES                # nodes per core
    ntile = _ceil(npc, P)                  # dst tiles per core
    core_of = dst // npc

    pc = []
    for c in range(NCORES):
        m = core_of == c
        es = src[m].astype(np.int64)
        ed = dst[m].astype(np.int64) - c * npc
        o = np.argsort(ed, kind="stable")
        es, ed = es[o], ed[o]
        bounds = np.searchsorted(ed, np.arange(0, ntile * P + 1, P))
        tiles = []
        for t in range(ntile):
            sl = slice(bounds[t], bounds[t + 1])
            ts_, td_ = es[sl], ed[sl]
            am = ts_ < SPLIT
            tiles.append(dict(
                a_src=ts_[am], a_fd=td_[am], a_col=td_[am] - t * P,
                b_src=ts_[~am] - SPLIT, b_fd=td_[~am], b_col=td_[~am] - t * P,
            ))
        pc.append(tiles)

    cA = [max(_ceil(len(pc[c][t]["a_src"]), P) for c in range(NCORES))
          for t in range(ntile)]
    cB = [max(_ceil(len(pc[c][t]["b_src"]), P) for c in range(NCORES))
          for t in range(ntile)]
    C = [cA[t] + cB[t] for t in range(ntile)]

    per_core = []
    for c in range(NCORES):
        iA, iB, iF, Ss = [], [], [], []
        for t in range(ntile):
            d = pc[c][t]
            nA, nB = len(d["a_src"]), len(d["b_src"])
            if cA[t]:
                iA.append(_wrap_idx(_pad(d["a_src"], cA[t] * P)))
            if cB[t]:
                iB.append(_wrap_idx(_pad(d["b_src"], cB[t] * P)))
            S3 = np.zeros((C[t] * P, P), dtype=np.float32)
            S3[np.arange(nA), d["a_col"]] = 1.0
            S3[cA[t] * P + np.arange(nB), d["b_col"]] = 1.0
            Ss.append(
                S3.reshape(C[t], P, P).transpose(1, 0, 2).reshape(P, C[t] * P))
            iF.append(
                S3.reshape(C[t], P, P).transpose(2, 0, 1).reshape(P, C[t] * P))
        per_core.append(dict(
            iA=np.concatenate(iA, axis=1) if iA else np.zeros((P, 8), np.int16),
            iB=np.concatenate(iB, axis=1) if iB else np.zeros((P, 8), np.int16),
            ST=np.concatenate(iF, axis=1).astype(_bf),
            S=np.concatenate(Ss, axis=1).astype(_bf),
        ))
    return per_core, cA, cB, C, ntile, npc


def _aug(w, b):
    return np.vstack([np.asarray(w), np.asarray(b)[None, :]])


def build_nc(meta, stop_after="full"):
    n_nodes = meta["n_nodes"]
    npc = meta["npc"]
    ntile = meta["ntile"]
    cA, cB, C = meta["cA"], meta["cB"], meta["C"]
    Cmax = max(C)
    sumA, sumB, sumC = sum(cA), sum(cB), sum(C)
    emb = meta["emb"]
    dd = meta["d"]
    H = meta["H"]
    F = dd // H
    ncls = meta["ncls"]
    npc_pad = ntile * P
    nfull = meta["nfull_pad"]
    rowsA = min(nfull, SPLIT)
    rowsB_pad = max(_ceil(nfull - rowsA, P) * P, P)
    ngrp = npc // TOPK
    use_B = n_nodes > SPLIT
    nfull_ag_pad = _ceil(n_nodes, P) * P

    nc = bacc.Bacc(num_swdge_queues=4)

    xTl = nc.declare_dram_parameter("xTl", [emb + 1, npc_pad], bf16, isOutput=False)
    w1s = nc.declare_dram_parameter("w1s", [emb + 1, dd], bf16, isOutput=False)
    w1d = nc.declare_dram_parameter("w1d", [emb + 1, dd], bf16, isOutput=False)
    w2s = nc.declare_dram_parameter("w2s", [dd + 1, dd], bf16, isOutput=False)
    w2d = nc.declare_dram_parameter("w2d", [dd + 1, dd], bf16, isOutput=False)
    a1r = nc.declare_dram_parameter("a1r", [P, dd], bf16, isOutput=False)
    a2r = nc.declare_dram_parameter("a2r", [P, dd], bf16, isOutput=False)
    wca = nc.declare_dram_parameter("wca", [dd + 1, ncls], f32, isOutput=False)
    pwp = nc.declare_dram_parameter("pwp", [P, 5], f32, isOutput=False)
    iA_in = nc.declare_dram_parameter("iA", [P, max(sumA, 1) * 8], i16, isOutput=False)
    iB_in = nc.declare_dram_parameter("iB", [P, max(sumB, 1) * 8], i16, isOutput=False)
    ST_in = nc.declare_dram_parameter("ST", [P, sumC * P], bf16, isOutput=False)
    S_in = nc.declare_dram_parameter("S", [P, sumC * P], bf16, isOutput=False)
    out = nc.declare_dram_parameter("out", [ngrp, ncls], f32, isOutput=True)

    fd1 = nc.dram_tensor("fd1", [npc_pad, dd], bf16)
    fd2 = nc.dram_tensor("fd2", [npc_pad, dd], bf16)
    fs1l = nc.dram_tensor("fs1l", [npc, dd], bf16)
    fs1f = nc.dram_tensor("fs1f", [nfull_ag_pad, dd], bf16, addr_space="Shared")
    fs2l = nc.dram_tensor("fs2l", [npc, dd], bf16)
    fs2f = nc.dram_tensor("fs2f", [nfull_ag_pad, dd], bf16, addr_space="Shared")

    AF = mybir.ActivationFunctionType
    ALU = mybir.AluOpType
    BLK = 1024

    with tile.TileContext(nc) as tc:
        with (
            tc.tile_pool(name="const", bufs=1) as cpool,
            tc.tile_pool(name="wpool", bufs=1) as wpool,
            tc.tile_pool(name="xload", bufs=2) as xpool,
            tc.tile_pool(name="mmout", bufs=3) as mpool,
            tc.tile_pool(name="edgeg", bufs=3) as epool,
            tc.tile_pool(name="vp", bufs=2) as vpool,
            tc.tile_pool(name="sp2", bufs=2) as s2pool,
            tc.tile_pool(name="zp", bufs=1) as zpool,
            tc.tile_pool(name="small", bufs=3) as spool,
            tc.tile_pool(name="hbuf", bufs=1) as hpool,
            tc.tile_pool(name="psA", bufs=2, space="PSUM") as psA,
            tc.tile_pool(name="psT", bufs=2, space="PSUM") as psT,
            tc.tile_pool(name="psE", bufs=2, space="PSUM") as psE,
            tc.tile_pool(name="psF", bufs=2, space="PSUM") as psF,
        ):
            ones1 = cpool.tile([1, P], bf16)
            nc.vector.memset(ones1[:], 1.0)
            ones1f = cpool.tile([1, P], f32)
            nc.vector.memset(ones1f[:], 1.0)
            ident = cpool.tile([P, P], bf16)
            make_identity(nc, ident[:])
            a1t = cpool.tile([P, dd], bf16)
            nc.sync.dma_start(out=a1t[:], in_=a1r[:])
            a2t = cpool.tile([P, dd], bf16)
            nc.sync.dma_start(out=a2t[:], in_=a2r[:])
            pw = cpool.tile([P, 5], f32)
            nc.sync.dma_start(out=pw[:], in_=pwp[:])

            def load_w(src_t, kdim, nm):
                t0 = wpool.tile([P, dd], bf16, tag=nm + "0")
                t1 = wpool.tile([P, dd], bf16, tag=nm + "1")
                t2 = wpool.tile([1, dd], bf16, tag=nm + "2")
                nc.sync.dma_start(out=t0[:], in_=src_t[0:P, :])
                nc.sync.dma_start(out=t1[:], in_=src_t[P:2 * P, :])
                nc.sync.dma_start(out=t2[:], in_=src_t[kdim:kdim + 1, :])
                return t0, t1, t2

            w1s_t = load_w(w1s, emb, "w1s")
            w1d_t = load_w(w1d, emb, "w1d")
            w2s_t = load_w(w2s, dd, "w2s")
            w2d_t = load_w(w2d, dd, "w2d")

            def mm_rows(x0, x1, m0, wtile, psum):
                nc.tensor.matmul(out=psum[:], lhsT=x0[:, m0:m0 + P],
                                 rhs=wtile[0][:], start=True, stop=False)
                nc.tensor.matmul(out=psum[:], lhsT=x1[:, m0:m0 + P],
                                 rhs=wtile[1][:], start=False, stop=False)
                nc.tensor.matmul(out=psum[:], lhsT=ones1[:], rhs=wtile[2][:],
                                 start=False, stop=True)

            # ---------- phase A: fs1 local -> AllGather, fd1 local overlaps --
            def lin_phase(wtile, dst_t, dst_rows):
                for b in range(_ceil(npc_pad, BLK)):
                    w = min(BLK, npc_pad - b * BLK)
                    x0 = xpool.tile([P, BLK], bf16, tag="x0")
                    x1 = xpool.tile([P, BLK], bf16, tag="x1")
                    nc.sync.dma_start(out=x0[:, :w],
                                      in_=xTl[0:P, b * BLK:b * BLK + w])
                    nc.sync.dma_start(out=x1[:, :w],
                                      in_=xTl[P:2 * P, b * BLK:b * BLK + w])
                    for m in range(w // P):
                        row0 = b * BLK + m * P
                        ps = psA.tile([P, dd], f32, space="PSUM", tag="psa")
                        mm_rows(x0, x1, m * P, wtile, ps)
                        ot = mpool.tile([P, dd], bf16, tag="fsout")
                        nc.scalar.copy(out=ot[:], in_=ps[:])
                        hi = min(row0 + P, dst_rows)
                        if hi > row0:
                            nc.sync.dma_start(out=dst_t[row0:hi, :],
                                              in_=ot[: hi - row0, :])

            lin_phase(w1s_t, fs1l, npc)
            lin_phase(w1d_t, fd1, npc_pad)
            nc.gpsimd.collective_compute(
                "AllGather", ALU.bypass,
                replica_groups=[list(range(NCORES))],
                ins=[fs1l[:]], outs=[fs1f[0:n_nodes, :]])

            # ---------- edge phase ----------
            h1 = hpool.tile([P, ntile, dd], bf16, tag="h")
            h2 = hpool.tile([P, ntile, dd], bf16, tag="h")

            def edge_phase(tabA, tabB, tabF, a_t, hdst, scale_posw, depth="all"):
                offA = offB = offC = 0
                for t in range(ntile):
                    ca, cb, ct = cA[t], cB[t], C[t]
                    St = s2pool.tile([P, Cmax * P], bf16, tag="S")
                    nc.sync.dma_start(
                        out=St[:, : ct * P],
                        in_=S_in[:, offC * P:(offC + ct) * P])
                    E = epool.tile([P, Cmax, dd], bf16, tag="E")
                    if ca:
                        ia = spool.tile([P, Cmax * 8], i16, tag="ia")
                        nc.sync.dma_start(
                            out=ia[:, : ca * 8],
                            in_=iA_in[:, offA * 8:(offA + ca) * 8])
                        ah = (ca + 1) // 2
                        nc.gpsimd.dma_gather(
                            out_ap=E[:, 0:ah, :], in_ap=tabA,
                            idxs_ap=ia[:, : ah * 8], num_idxs=ah * P,
                            num_idxs_reg=ah * P, elem_size=dd,
                            single_packet=False, queue_num=(3 * t) % 4)
                        if ca > ah:
                            nc.gpsimd.dma_gather(
                                out_ap=E[:, ah:ca, :], in_ap=tabA,
                                idxs_ap=ia[:, ah * 8: ca * 8],
                                num_idxs=(ca - ah) * P,
                                num_idxs_reg=(ca - ah) * P, elem_size=dd,
                                single_packet=False, queue_num=(3 * t + 1) % 4)
                    if cb:
                        ib = spool.tile([P, Cmax * 8], i16, tag="ib")
                        nc.sync.dma_start(
                            out=ib[:, : cb * 8],
                            in_=iB_in[:, offB * 8:(offB + cb) * 8])
                        nc.gpsimd.dma_gather(
                            out_ap=E[:, ca:ct, :], in_ap=tabB,
                            idxs_ap=ib[:, : cb * 8], num_idxs=cb * P,
                            num_idxs_reg=cb * P, elem_size=dd,
                            single_packet=False, queue_num=(3 * t + 2) % 4)
                    # fd broadcast: fd rows for this tile's 128 dsts, expanded
                    # to edge slots via the transposed one-hot (PE matmul)
                    STt = s2pool.tile([P, Cmax * P], bf16, tag="ST")
                    nc.sync.dma_start(
                        out=STt[:, : ct * P],
                        in_=ST_in[:, offC * P:(offC + ct) * P])
                    fdt = spool.tile([P, dd], bf16, tag="fdt")
                    nc.sync.dma_start(out=fdt[:], in_=tabF[t * P:(t + 1) * P, :])
                    w_ = ct * dd
                    LZ = zpool.tile([P, Cmax * dd], bf16, tag="LZ")
                    G = 2
                    for g0 in range(0, ct, G):
                        gn = min(G, ct - g0)
                        psf = psF.tile([P, G, dd], f32, space="PSUM", tag="psf")
                        for i in range(gn):
                            nc.tensor.matmul(
                                out=psf[:, i, :],
                                lhsT=STt[:, (g0 + i) * P:(g0 + i + 1) * P],
                                rhs=fdt[:], start=True, stop=False)
                            nc.tensor.matmul(
                                out=psf[:, i, :], lhsT=ident[:],
                                rhs=E[:, g0 + i, :],
                                start=False, stop=True)
                        nc.scalar.activation(
                            LZ[:, g0 * dd:(g0 + gn) * dd].rearrange(
                                "p (g d) -> p g d", d=dd),
                            psf[:, 0:gn, :], AF.Prelu, alpha=NEG_SLOPE)

                    if depth == "g":
                        nc.vector.memset(hdst[:, t, :], 0.0)
                        offA += ca; offB += cb; offC += ct
                        continue
                    T = zpool.tile([P, Cmax * dd], bf16, tag="T")
                    nc.vector.tensor_mul(
                        out=T[:, :w_].rearrange("p (c d) -> p c d", d=dd),
                        in0=LZ[:, :w_].rearrange("p (c d) -> p c d", d=dd),
                        in1=_bcast_mid(a_t[:], ct))
                    TF = zpool.tile([P, Cmax * dd // 2], bf16, tag="TF")
                    tv = T[:, :w_].rearrange("p (ch f) -> p ch f", f=F)
                    nc.vector.tensor_add(
                        out=TF[:, : w_ // 2].rearrange("p (ch f) -> p ch f",
                                                       f=F // 2),
                        in0=tv[:, :, 0:F // 2], in1=tv[:, :, F // 2:F])
                    score = spool.tile([P, Cmax * H], f32, tag="sc")
                    nc.vector.reduce_sum(
                        out=score[:, : ct * H],
                        in_=TF[:, : w_ // 2].rearrange("p (ch f) -> p ch f",
                                                       f=F // 2),
                        axis=mybir.AxisListType.X)
                    EX = spool.tile([P, Cmax * H], f32, tag="ex")
                    nc.scalar.activation(EX[:, : ct * H], score[:, : ct * H],
                                         AF.Exp)
                    if depth == "dve":
                        nc.vector.memset(hdst[:, t, :], 0.0)
                        offA += ca; offB += cb; offC += ct
                        continue

                    V = vpool.tile([P, Cmax, dd + H], bf16, tag="V")
                    EXB = zpool.tile([P, Cmax * dd], bf16, tag="EXB")
                    exs = EX[:, : ct * H]
                    nc.scalar.copy(
                        out=EXB[:, :w_].rearrange("p (ch f) -> p ch f", f=F),
                        in_=AP(exs.tensor, exs.offset,
                               [exs.ap[0], exs.ap[1], [0, F]]))
                    nc.vector.tensor_mul(
                        out=V[:, 0:ct, 0:dd],
                        in0=E[:, 0:ct, :],
                        in1=EXB[:, :w_].rearrange("p (c d) -> p c d", d=dd))
                    nc.scalar.copy(
                        out=V[:, 0:ct, dd:dd + H],
                        in_=EX[:, : ct * H].rearrange("p (c h) -> p c h", h=H))

                    if depth == "v":
                        nc.vector.memset(hdst[:, t, :], 0.0)
                        offA += ca; offB += cb; offC += ct
                        continue
                    agg = psE.tile([P, dd + H], f32, space="PSUM", tag="agg")
                    for c in range(ct):
                        nc.tensor.matmul(
                            out=agg[:], lhsT=St[:, c * P:(c + 1) * P],
                            rhs=V[:, c, :], start=(c == 0), stop=(c == ct - 1))

                    den = spool.tile([P, H], f32, tag="den")
                    nc.vector.tensor_scalar_max(den[:], agg[:, dd:dd + H], 1e-9)
                    rec = spool.tile([P, H], f32, tag="rec")
                    nc.vector.reciprocal(rec[:], den[:])
                    if scale_posw:
                        nc.vector.tensor_scalar_mul(rec[:], rec[:],
                                                    pw[:, t % 5:t % 5 + 1])
                    for h in range(H):
                        nc.vector.tensor_scalar_mul(
                            hdst[:, t, h * F:(h + 1) * F],
                            agg[:, h * F:(h + 1) * F], rec[:, h:h + 1])
                    offA += ca
                    offB += cb
                    offC += ct

            if stop_after != "A":
                depth = {"L1g": "g", "L1dve": "dve", "L1v": "v"}.get(
                    stop_after, "all")
                if use_B:
                    edge_phase(fs1f[0:SPLIT, :], fs1f[SPLIT:nfull_ag_pad, :],
                               fd1, a1t, h1, False, depth)
                else:
                    edge_phase(fs1f[0:nfull_ag_pad, :], fs1f[:],
                               fd1, a1t, h1, False, depth)

            # ---------- transpose h1 ----------
            hT0 = hpool.tile([P, npc_pad], bf16, tag="t0")
            hT1 = hpool.tile([P, npc_pad], bf16, tag="t1")
            for t in range(ntile if stop_after in ("TR", "FS2", "AG", "full") else 0):
                for half, ht in ((0, hT0), (1, hT1)):
                    pt = psT.tile([P, P], bf16, space="PSUM", tag="ptr")
                    nc.tensor.transpose(
                        out=pt[:], in_=h1[:, t, half * P:(half + 1) * P],
                        identity=ident[:])
                    nc.scalar.copy(out=ht[:, t * P:(t + 1) * P], in_=pt[:])

            # ---------- fs2 local -> AllGather, fd2 overlaps ----------
            do_l2 = stop_after in ("FS2", "AG", "full")

            def lin2_phase(wt, dst_t, dst_rows):
                for t in range(ntile):
                    ps = psA.tile([P, dd], f32, space="PSUM", tag="psa")
                    nc.tensor.matmul(out=ps[:], lhsT=hT0[:, t * P:(t + 1) * P],
                                     rhs=wt[0][:], start=True, stop=False)
                    nc.tensor.matmul(out=ps[:], lhsT=hT1[:, t * P:(t + 1) * P],
                                     rhs=wt[1][:], start=False, stop=False)
                    nc.tensor.matmul(out=ps[:], lhsT=ones1[:], rhs=wt[2][:],
                                     start=False, stop=True)
                    ot = mpool.tile([P, dd], bf16, tag="fsout")
                    nc.scalar.copy(out=ot[:], in_=ps[:])
                    hi = min((t + 1) * P, dst_rows)
                    if hi > t * P:
                        nc.sync.dma_start(out=dst_t[t * P:hi, :],
                                          in_=ot[: hi - t * P, :])

            if do_l2:
                lin2_phase(w2s_t, fs2l, npc)
                lin2_phase(w2d_t, fd2, npc_pad)

            # ---------- AllGather fs2 ----------
            do_rest = stop_after in ("AG", "full")
            if do_rest:
                nc.gpsimd.collective_compute(
                "AllGather", ALU.bypass,
                    replica_groups=[list(range(NCORES))],
                    ins=[fs2l[:]], outs=[fs2f[0:n_nodes, :]])

            if do_rest and stop_after == "AG":
                do_rest = False
            if do_rest and use_B:
                edge_phase(fs2f[0:SPLIT, :], fs2f[SPLIT:nfull_ag_pad, :],
                           fd2, a2t, h2, True)
            elif do_rest:
                edge_phase(fs2f[0:nfull_ag_pad, :], fs2f[:], fd2, a2t, h2, True)

            # ---------- transpose h2 ----------
            gT0 = hpool.tile([P, npc_pad], bf16, tag="t0")
            gT1 = hpool.tile([P, npc_pad], bf16, tag="t1")
            for t in range(ntile if stop_after == "full" else 0):
                for half, ht in ((0, gT0), (1, gT1)):
                    pt = psT.tile([P, P], bf16, space="PSUM", tag="ptr")
                    nc.tensor.transpose(
                        out=pt[:], in_=h2[:, t, half * P:(half + 1) * P],
                        identity=ident[:])
                    nc.scalar.copy(out=ht[:, t * P:(t + 1) * P], in_=pt[:])

            # ---------- pooling + classifier ----------
            p0 = hpool.tile([P, npc // TOPK], f32, tag="p0")
            p1 = hpool.tile([P, npc // TOPK], f32, tag="p1")
            if stop_after == "full":
                nc.vector.reduce_sum(
                    out=p0[:],
                    in_=gT0[:, :npc].rearrange("p (g k) -> p g k", k=TOPK),
                    axis=mybir.AxisListType.X)
                nc.vector.reduce_sum(
                    out=p1[:],
                    in_=gT1[:, :npc].rearrange("p (g k) -> p g k", k=TOPK),
                    axis=mybir.AxisListType.X)
            else:
                nc.vector.memset(p0[:], 0.0)
                nc.vector.memset(p1[:], 0.0)

            wc0 = wpool.tile([P, ncls], f32, tag="wc0")
            wc1 = wpool.tile([P, ncls], f32, tag="wc1")
            wc2 = wpool.tile([1, ncls], f32, tag="wc2")
            nc.sync.dma_start(out=wc0[:], in_=wca[0:P, :])
            nc.sync.dma_start(out=wc1[:], in_=wca[P:2 * P, :])
            nc.sync.dma_start(out=wc2[:], in_=wca[dd:dd + 1, :])

            for g0 in range(0, ngrp, P):
                gw = min(P, ngrp - g0)
                pc_ = psA.tile([P, ncls], f32, space="PSUM", tag="psa")
                nc.tensor.matmul(out=pc_[:gw, :], lhsT=p0[:, g0:g0 + gw],
                                 rhs=wc0[:], start=True, stop=False)
                nc.tensor.matmul(out=pc_[:gw, :], lhsT=p1[:, g0:g0 + gw],
                                 rhs=wc1[:], start=False, stop=False)
                nc.tensor.matmul(out=pc_[:gw, :], lhsT=ones1f[:, :gw],
                                 rhs=wc2[:], start=False, stop=True)
                oc = mpool.tile([P, ncls], f32, tag="ocls")
                nc.scalar.copy(out=oc[:gw, :], in_=pc_[:gw, :])
                nc.sync.dma_start(out=out[g0:g0 + gw, :], in_=oc[:gw, :])

    nc.compile()
    return nc


def _build_inputs(inputs):
    x = np.asarray(inputs["x"], dtype=np.float32)
    src = np.asarray(inputs["src"]).astype(np.int64)
    dst = np.asarray(inputs["dst"]).astype(np.int64)
    n_nodes, emb = x.shape
    dd = np.asarray(inputs["w1_src"]).shape[1]
    H = np.asarray(inputs["a1"]).shape[0]
    ncls = np.asarray(inputs["wc"]).shape[1]

    per_core, cA, cB, C, ntile, npc = _prep_graph(src, dst, n_nodes)
    npc_pad = ntile * P
    nfull_pad = _ceil(n_nodes, P) * P

    meta = dict(n_nodes=n_nodes, npc=npc, ntile=ntile, cA=cA, cB=cB, C=C,
                emb=emb, d=dd, H=H, ncls=ncls, nfull_pad=nfull_pad)

    w1s = _aug(inputs["w1_src"], inputs["b1_src"]).astype(_bf)
    w1d = _aug(inputs["w1_dst"], inputs["b1_dst"]).astype(_bf)
    w2s = _aug(inputs["w2_src"], inputs["b2_src"]).astype(_bf)
    w2d = _aug(inputs["w2_dst"], inputs["b2_dst"]).astype(_bf)
    a1rr = np.tile(np.asarray(inputs["a1"]).reshape(1, -1), (P, 1)).astype(_bf)
    a2rr = np.tile(np.asarray(inputs["a2"]).reshape(1, -1), (P, 1)).astype(_bf)
    wca = _aug(inputs["wc"], inputs["bc"]).astype(np.float32)
    pos_w = np.asarray(inputs["pos_w"], dtype=np.float32)
    pwp = np.zeros((P, 5), dtype=np.float32)
    for j in range(5):
        for p in range(P):
            pwp[p, j] = pos_w[(P * j + p) % TOPK]

    in_maps = []
    for c in range(NCORES):
        d = per_core[c]
        xl = np.zeros((emb + 1, npc_pad), dtype=_bf)
        xl[:emb, :npc] = x[c * npc:(c + 1) * npc].T.astype(_bf)
        xl[emb, :] = _bf(1.0)
        in_maps.append(dict(
            xTl=xl, w1s=w1s, w1d=w1d, w2s=w2s, w2d=w2d,
            a1r=a1rr, a2r=a2rr, wca=wca, pwp=pwp,
            iA=np.ascontiguousarray(d["iA"]), iB=np.ascontiguousarray(d["iB"]),
            ST=np.ascontiguousarray(d["ST"]), S=np.ascontiguousarray(d["S"]),
        ))
    return meta, in_maps


def run(inputs, trace=False, stop_after="full", cores=None):
    meta, in_maps = _build_inputs(inputs)
    nc = build_nc(meta, stop_after=stop_after)
    ids = list(range(NCORES)) if cores is None else list(range(cores))
    res = run_bass_kernel_spmd(nc, [in_maps[c] for c in ids], core_ids=ids,
                               trace=trace)
    outs = [res.results[i]["out"] for i in range(len(ids))]
    return np.concatenate(outs, axis=0), res


def kernel(**inputs):
    out, _ = run(inputs, trace=False)
    return out

